# revision 1
# baseline (speedup 1.0000x reference)
"""AdaptiveESN Trainium2 kernel.

Echo State Network: B=64, T=2048, D=128, H=512, leaky a=0.26.
    h_t = (1-a) h_{t-1} + a tanh(x_t W_in^T + b_in + h_{t-1} W_res^T + b_res)
    y_t = h_t W_ro^T

Strategy: TIME-parallel across 8 NeuronCores, with G_PER_CORE=4 independent
time segments interleaved per core as extra rhs columns (32 segments total,
rhs = 256 columns/step). The per-step PE cost is a stream of 20
LDWEIGHTS+MATMUL pairs; LDWEIGHTS does not hide behind short matmuls, so
widening the rhs amortizes it (measured per-scan: g=1 ~450, g=2 ~343,
g=4 ~327, g=8 ~378 us bf16; fp8 W_res cuts LDWEIGHTS ~2x -> 239 us at g=4).
The ESN is contracting (||(1-a)I + a D W_res||_2 <= 0.74 + 0.26*0.18 ~ 0.79),
so every segment except the first recovers the true state of its segment
start by running BURN=16 extra steps from h=0 (state error ~1e-4). Segment 0
starts at t=0 exactly. Per core: S = 2048/32 + BURN = 80 steps.

Per step the scan is the batched matvec z = (2^8 a W_res) h~ + (2^8 W_in) x_t
with h~ = h/a (leak scale folded into the weights, 2^8 prescale keeps the
fp8e4m3 W_res out of the subnormal range; the tanh activation applies input
scale 2^-8 and the f32 bias). Blend h~_new = (1-a) h~ + tanh(...) is ONE
fused DVE op per chunk (scalar_tensor_tensor). Within a step the 20 matmul
pairs are ordered by rhs availability (x-consuming pairs first, then h
chunks 0..3) so the previous step's tanh/blend chain has maximal slack
(measured ~25% faster than per-out-chunk grouping). Readout y = (a W_ro) h~
is fused per time-chunk.

Layouts (host-prepped, per core c; segment q = c*G+e, t0_q = max(0, q*SEG-BURN)):
    xt   bf16 [128, S*BL]  xt[d, s*BL + e*64 + b] = x[b, t0 + s, d]
    wres fp8  [128, 2048]  tile (j,i) at cols (j*4+i)*128: (2^8 a W_res).T block
    win  bf16 [128, 512]   (2^8 W_in).T
    wro  bf16 [128, 512]   tile j at cols j*128: (a W_ro).T block
    bias f32  [128, 4]     (b_in + b_res) chunk i in col i
    out  f32  [128, S*BL]  out[d, s*BL + e*64 + b] = y[b, t0 + s, d]
Host gather keeps steps [0,SEG) for segment 0, [BURN,S) otherwise.
"""
import sys

if "/opt/trn_rl_repo" not in sys.path:
    sys.path.insert(0, "/opt/trn_rl_repo")

import numpy as np
import ml_dtypes

import concourse.bass as bass
from concourse import bacc
import concourse.mybir as mybir
import concourse.tile as tile
from concourse.bass_utils import run_bass_kernel_spmd

try:
    import jax

    jax.config.update("jax_compilation_cache_dir", "/tmp/jax_neff_cache")
    jax.config.update("jax_persistent_cache_min_compile_time_secs", 10)
except Exception:
    pass

B, T, D, H = 64, 2048, 128, 512
LEAKY = 0.26
NCORES = 8
NCH = H // 128            # H chunks (partition tiles)
BF16 = mybir.dt.bfloat16
F32 = mybir.dt.float32


def configure(g=4, burn=16, tc=20):
    """Set the segment layout. g = independent time segments interleaved per
    core (rhs columns per step = 64*g), burn = burn-in steps for segments > 0,
    tc = time steps per states chunk (S/tc chunks must be even for reps)."""
    global G_PER_CORE, NSEG, SEG, BURN, S, BL, TC, W
    G_PER_CORE = g
    NSEG = NCORES * g
    SEG = T // NSEG           # output steps per segment
    BURN = burn
    S = SEG + BURN            # scan steps per core
    BL = B * g                # rhs columns per step (segments x batch)
    TC = tc
    W = NCH * BL              # step-major state width
    assert (S // tc) * tc == S


import os as _os

if _os.environ.get("KCFG"):
    configure(*[int(v) for v in _os.environ["KCFG"].split(",")])
else:
    configure()

TRACE = False             # test harness can flip this for profiling
WRES_FP8 = True           # fp8e4m3 W_res with 2^8 prescale (halves LDWEIGHTS time)
FP8_SCALE = 256.0
_last_results = None      # BassKernelResults of the most recent run


def build(s_total=None, tc=None, reps=1, wres_fp8=WRES_FP8, order="x_first", probe=None,
          blend="per_chunk", ps_split=(5, 3)):
    """Build the per-core Bacc graph (same graph on all 8 cores).

    reps > 1 wraps the whole scan in a hardware For_i loop that re-runs it
    (same instruction count) — used to measure pure on-device time via
    wall-clock deltas between two reps values.

    order: matmul pair order within a step.
      "grouped" - 5 consecutive pairs per out-chunk into one psum bank
                  (order j0,j1,j2,win,j3 — defer the h[3] consumption)
      "x_first" - by rhs availability: win x4, then h0 x4 .. h3 x4
                  (psum bank cycles every MM)
    probe: timing-only structural variants (WRONG math, never for output):
      "noro"  - skip the readout matmuls/copies/DMAs
      "nodep" - matmul rhs reads the zero tile (h0) instead of the state:
                breaks the cross-step serial chain, same engine load
      "nepi"  - nodep AND no ACT/tanh/blend at all (pure PE burst rate)
      "nepi12"- nepi with only 3 pairs per chunk (12 vs 20: marginal pair cost)
    """
    s_total = S if s_total is None else s_total
    tc = TC if tc is None else tc
    nchunks = s_total // tc
    assert nchunks * tc == s_total
    assert nchunks % 2 == 0 or nchunks == 1 or reps == 1

    nc = bacc.Bacc(None, target_bir_lowering=False)
    xt_e = nc.declare_dram_parameter("xt", [128, s_total * BL], BF16, isOutput=False)
    wres_dt = mybir.dt.float8e4 if wres_fp8 else BF16
    wres_e = nc.declare_dram_parameter("wres", [128, 16 * 128], wres_dt, isOutput=False)
    win_e = nc.declare_dram_parameter("win", [128, NCH * 128], BF16, isOutput=False)
    wro_e = nc.declare_dram_parameter("wro", [128, NCH * 128], BF16, isOutput=False)
    bias_e = nc.declare_dram_parameter("bias", [128, NCH], F32, isOutput=False)
    out_e = nc.declare_dram_parameter("out", [128, s_total * BL], F32, isOutput=True)
    act_scale = (1.0 / FP8_SCALE) if wres_fp8 else 1.0

    with tile.TileContext(nc) as tc_ctx:
        with (
            tc_ctx.tile_pool(name="const", bufs=1) as const_pool,
            tc_ctx.tile_pool(name="p", bufs=8) as p_pool,
            tc_ctx.tile_pool(name="ostage", bufs=3) as o_pool,
            tc_ctx.tile_pool(name="scan_ps", bufs=ps_split[0], space=bass.MemorySpace.PSUM) as ps_pool,
            tc_ctx.tile_pool(name="ro_ps", bufs=ps_split[1], space=bass.MemorySpace.PSUM) as ro_pool,
        ):
            xt_sb = const_pool.tile([128, s_total * BL], BF16)
            wres_sb = const_pool.tile([128, 16 * 128], wres_dt)
            win_sb = const_pool.tile([128, NCH * 128], BF16)
            wro_sb = const_pool.tile([128, NCH * 128], BF16)
            bias_sb = const_pool.tile([128, NCH], F32)
            h0_sb = const_pool.tile([128, W], BF16)
            # states, step-major: column s*W + i*BL + col  (i = H chunk)
            st = [
                const_pool.tile([128, tc * W], BF16, name=f"st{k}", tag=f"st{k}")
                for k in range(2)
            ]

            nc.sync.dma_start(wres_sb[:], wres_e[:])
            nc.sync.dma_start(win_sb[:], win_e[:])
            nc.sync.dma_start(wro_sb[:], wro_e[:])
            nc.sync.dma_start(bias_sb[:], bias_e[:])
            nc.sync.dma_start(xt_sb[:], xt_e[:])
            nc.vector.memset(h0_sb[:], 0.0)
            if probe in ("nepi", "nepi12"):
                nc.vector.memset(st[0][:], 0.0)
                nc.vector.memset(st[1][:], 0.0)

            def scan_body(_iv=None):
                for c in range(nchunks):
                    cur, prv = c % 2, (c - 1) % 2
                    for s in range(tc):
                        t = c * tc + s
                        if t == 0:
                            hprev = h0_sb[:]
                        elif s == 0:
                            hprev = st[prv][:, (tc - 1) * W : tc * W]
                        else:
                            hprev = st[cur][:, (s - 1) * W : s * W]

                        no_dep = probe in ("nodep", "nepi", "nepi12")
                        no_epi = probe in ("nepi", "nepi12")
                        npairs = 3 if probe == "nepi12" else 5

                        def hcol(j):
                            if no_dep:
                                return h0_sb[:, j * BL : (j + 1) * BL]
                            return hprev[:, j * BL : (j + 1) * BL]

                        xcol = xt_sb[:, t * BL : (t + 1) * BL]
                        ps = [ps_pool.tile([128, BL], F32, name="ps") for _ in range(NCH)]

                        def wres_t(j, i):
                            return wres_sb[:, (j * NCH + i) * 128 : (j * NCH + i + 1) * 128]

                        def win_t(i):
                            return win_sb[:, i * 128 : (i + 1) * 128]

                        if order == "x_first":
                            # by rhs availability: win x4, then h0..h3 x4 each
                            pairs = [(i, win_t(i), xcol) for i in range(NCH)] + [
                                (i, wres_t(j, i), hcol(j))
                                for j in range(NCH)
                                for i in range(NCH)
                            ]
                        else:
                            # 5 consecutive pairs per out-chunk into one psum bank
                            pairs = [
                                (i, lhsT, rhs)
                                for i in range(NCH)
                                for (lhsT, rhs) in [
                                    (wres_t(0, i), hcol(0)),
                                    (wres_t(1, i), hcol(1)),
                                    (wres_t(2, i), hcol(2)),
                                    (win_t(i), xcol),
                                    (wres_t(3, i), hcol(3)),
                                ]
                            ]
                        if npairs == 3:
                            pairs = [p for k, p in enumerate(pairs)
                                     if (k % 5 < 3 if order != "x_first" else k < 12)]
                        started, count = set(), {}
                        per_chunk = npairs
                        for i, lhsT, rhs in pairs:
                            count[i] = count.get(i, 0) + 1
                            nc.tensor.matmul(
                                ps[i][:], lhsT, rhs,
                                start=(i not in started),
                                stop=(count[i] == per_chunk))
                            started.add(i)
                        if not no_epi:
                            if blend == "fat":
                                # one blend + one semaphore per step: all 16
                                # h-consuming matmuls of step t+1 wait on it
                                p_t = p_pool.tile([128, W], BF16, name="p")
                                for i in range(NCH):
                                    nc.scalar.activation(
                                        p_t[:, i * BL : (i + 1) * BL], ps[i][:],
                                        mybir.ActivationFunctionType.Tanh,
                                        bias=bias_sb[:, i : i + 1], scale=act_scale,
                                    )
                                nc.vector.scalar_tensor_tensor(
                                    st[cur][:, s * W : (s + 1) * W],
                                    hprev,
                                    1.0 - LEAKY,
                                    p_t[:],
                                    op0=mybir.AluOpType.mult,
                                    op1=mybir.AluOpType.add,
                                )
                            else:
                                for i in range(NCH):
                                    st_col = st[cur][:, s * W + i * BL : s * W + (i + 1) * BL]
                                    p_t = p_pool.tile([128, BL], BF16, name="p")
                                    nc.scalar.activation(
                                        p_t[:], ps[i][:], mybir.ActivationFunctionType.Tanh,
                                        bias=bias_sb[:, i : i + 1], scale=act_scale,
                                    )
                                    nc.vector.scalar_tensor_tensor(
                                        st_col,
                                        hprev[:, i * BL : (i + 1) * BL],
                                        1.0 - LEAKY,
                                        p_t[:],
                                        op0=mybir.AluOpType.mult,
                                        op1=mybir.AluOpType.add,
                                    )
                    if probe in ("noro", "nepi", "nepi12"):
                        continue
                    # fused readout of chunk c: out = (a W_ro).T @ states
                    base = c * tc * BL
                    st_v = st[cur].rearrange("p (s w) -> p s w", w=W)
                    ns = 512 // BL  # steps per readout tile
                    for n in range(0, tc, ns):
                        nw = min(ns, tc - n)
                        rps = ro_pool.tile([128, 512], F32)
                        for j in range(NCH):
                            nc.tensor.matmul(
                                rps[:, : nw * BL],
                                wro_sb[:, j * 128 : (j + 1) * 128],
                                st_v[:, n : n + nw, j * BL : (j + 1) * BL],
                                start=(j == 0),
                                stop=(j == NCH - 1),
                            )
                        ostage = o_pool.tile([128, 512], F32)
                        nc.scalar.activation(
                            ostage[:, : nw * BL], rps[:, : nw * BL],
                            mybir.ActivationFunctionType.Copy,
                        )
                        nc.sync.dma_start(
                            out_e[:, base + n * BL : base + (n + nw) * BL],
                            ostage[:, : nw * BL],
                        )

            if reps == 1:
                scan_body()
            else:
                with tc_ctx.For_i(0, reps, 1) as _i:
                    scan_body(_i)

    nc.compile()
    return nc


def _seg_t0(q):
    return 0 if q == 0 else q * SEG - BURN


def host_prep(x, W_in, b_in, W_res, b_res, W_ro, wres_fp8=WRES_FP8):
    """Produce the per-core in_maps (host-side layout/dtype prep only)."""
    a = np.float32(LEAKY)
    wscale = np.float32(FP8_SCALE) if wres_fp8 else np.float32(1.0)
    AT = (wscale * a * W_res).T.astype(np.float32)            # [in, out]
    wres_np_dt = ml_dtypes.float8_e4m3 if wres_fp8 else ml_dtypes.bfloat16
    wres = (
        AT.reshape(NCH, 128, NCH, 128).transpose(1, 0, 2, 3).reshape(128, 16 * 128)
    ).astype(wres_np_dt)
    win = (wscale * W_in).T.astype(ml_dtypes.bfloat16)        # [128, 512]
    R = (a * W_ro).T.astype(np.float32)                       # [512, 128]
    wro = R.reshape(NCH, 128, 128).transpose(1, 0, 2).reshape(128, NCH * 128).astype(
        ml_dtypes.bfloat16
    )
    bias = (b_in + b_res).astype(np.float32).reshape(NCH, 128).T.copy()  # [128, 4]

    in_maps = []
    for c in range(NCORES):
        segs = [x[:, _seg_t0(c * G_PER_CORE + e) :][:, :S, :] for e in range(G_PER_CORE)]
        xs = np.stack(segs, 0)                                # [g, 64, S, 128]
        xt = np.ascontiguousarray(xs.transpose(3, 2, 0, 1).reshape(128, S * BL))
        in_maps.append({
            "xt": xt.astype(ml_dtypes.bfloat16),
            "wres": wres, "win": win, "wro": wro, "bias": bias,
        })
    return in_maps


_nc_cache = {}


def kernel(x, W_in, b_in, W_res, b_res, W_ro):
    """Full inputs in, full output out ([B, T, D] float32)."""
    global _last_results
    x, W_in, b_in, W_res, b_res, W_ro = (
        np.asarray(t, dtype=np.float32) for t in (x, W_in, b_in, W_res, b_res, W_ro)
    )
    assert x.shape == (B, T, D)
    if "nc" not in _nc_cache:
        _nc_cache["nc"] = build()
    nc = _nc_cache["nc"]

    in_maps = host_prep(x, W_in, b_in, W_res, b_res, W_ro)
    try:
        res = run_bass_kernel_spmd(nc, in_maps, list(range(NCORES)), trace=TRACE)
    except Exception:
        # transient NRT_EXEC_UNIT_UNRECOVERABLE has been observed when a run
        # starts right on the heels of another process's teardown — retry once
        import time as _time

        _time.sleep(15)
        res = run_bass_kernel_spmd(nc, in_maps, list(range(NCORES)), trace=TRACE)
    _last_results = res

    out = np.empty((B, T, D), dtype=np.float32)
    for c in range(NCORES):
        oc = res.results[c]["out"].reshape(128, S, G_PER_CORE, B)  # [d, s, e, b]
        for e in range(G_PER_CORE):
            q = c * G_PER_CORE + e
            s0 = 0 if q == 0 else BURN
            out[:, q * SEG : (q + 1) * SEG] = oc[:, s0 : s0 + SEG, e].transpose(2, 1, 0)
    return out

